# revision 26
# baseline (speedup 1.0000x reference)
"""Llama GQA attention layer (B=1, S=2048, D=4096, H=32, KVH=8, DH=128) on 8 trn2 cores.

Sharding: tensor-parallel over heads. Core c owns Q heads [4c, 4c+4) and KV head c:
  Wq[:, c*512:(c+1)*512], Wk/Wv[:, c*128:(c+1)*128], Wo columns [c*512:(c+1)*512].

Data logistics are the dominant cost on this axon-tunneled setup (~55 MB/s to the
device), so the kernel minimizes host<->device bytes:
  - hidden_states / cos / sin / mask are SEQUENCE-SHARDED on the host (each core
    receives its 256-seq slice, bf16, split into three packed tensors) and
    AllGathered on device (three pipelined AGs so projections start after the
    first small one).
  - Wo is sharded COLUMN-wise; after attention, the per-head attention outputs
    (at, bf16) are AllGathered on device CHUNK-WISE (4 AGs overlapping phase B)
    so each core computes a DISJOINT [2048, 512] output slice. No host-side
    reduction, no fp32 partials: the output download is 8 x 2 MB bf16 instead
    of 8 x 32 MB fp32 (and the donated zero output buffers shrink identically).

Kernel layout strategy (per core):
  - X^T [4096, 2048] from the gathered blocks; projections computed as
    Q^T/K^T/V^T [dh, s] via PSUM accumulation over 32 d-tiles.
  - RoPE applied on PSUM evacuation (DVE, partition-half shuffle).
  - V^T transposed to V natural [s, dh] via PE-transpose (needed as PV stationary).
  - Attention with scores TRANSPOSED: S^T[k, q] tiles [128, 512] so softmax sums
    over keys become ones-vector matmuls; exp on ACT (no max subtraction - scores
    are O(10), exp is safe); causal sparsity by skipping fully-masked key tiles;
    diagonal tiles masked multiplicatively with 4 static 0/1 tiles.
  - Softmax normalization: recip of sums row [1,512] broadcast across partitions
    via a K=1 ones matmul, then one DVE mul per attn^T tile.
  - at stored chunk-major ([128, ci*2048 + h*512 + q]) so each chunk's AG input
    is one contiguous [128, 2048] slice; phase C walks (chunk, rank-block) with
    one 0.5 MB DMA per block and accumulates all 32 heads into 4 PSUM banks.
"""

import os as _os

import numpy as np

import concourse.bass as bass
import concourse.bacc as bacc
import concourse.mybir as mybir
import concourse.tile as tile
from concourse.bass_utils import run_bass_kernel_spmd

S = 2048
D = 4096
H = 32
KVH = 8
DH = 128
NCORES = 8
HPC = H // NCORES            # 4 query heads per core
QC = HPC * DH                # 512 projection cols per core
SCALE = float(DH) ** -0.5
NT_D = D // 128              # 32 contraction tiles
NCH = S // 512               # 4 sequence chunks
SPC = S // NCORES            # 256 seq positions shipped per core
FP32 = mybir.dt.float32
FP32R = mybir.dt.float32r
BF16 = mybir.dt.bfloat16
AF = mybir.ActivationFunctionType

MMDT = {"bf16": BF16, "fp32r": FP32R}[_os.environ.get("KERNEL_MM_DTYPE", "bf16")]

# per-core input split in three packed tensors (three pipelined AllGathers;
# phase A walks d-tiles in AG order across ALL four seq chunks, so matmuls
# start right after the first small AG and never starve on the later ones):
#   xa1: d-tiles 0..3                       [128,  4*SPC] = [128, 1024]
#   xa2: d-tiles 4..11                      [128,  8*SPC] = [128, 2048]
#   xcb: d-tiles 12..31 | cos | sin | mask  [128, 23*SPC] = [128, 5888]
N_A1 = 4
N_A2 = 8
XA1_W = N_A1 * SPC           # 1024
XA2_W = N_A2 * SPC           # 2048
XB_X = (NT_D - N_A1 - N_A2) * SPC   # 5120 (xt portion of xcb)
CS_O = XB_X                  # cos offset inside xcb's tail
SN_O = XB_X + SPC
MK_O = XB_X + 2 * SPC
XB_W = XB_X + 3 * SPC        # 5888
# d-tile quarters matching AG arrival order (each a run of dt-pairs)
QUARTERS = [range(0, N_A1), range(N_A1, N_A1 + N_A2),
            range(N_A1 + N_A2, 22), range(22, NT_D)]


def _np_mmdt():
    import ml_dtypes
    return {BF16: ml_dtypes.bfloat16, FP32R: np.float32}[MMDT]


def _emit(nc, tc, io, mode, phases="ABC"):
    """mode: 'causal' (sparse, static diag masks), 'dense' (all tiles, no mask),
    'masked' (all tiles, additive mask streamed from DRAM)."""
    from contextlib import ExitStack

    xa1_d, xa2_d, xcb_d, wq_d, wk_d, wv_d, wo_d, msk_d, id_d, on_d, out_d = io

    with ExitStack() as top:
        ep = top.enter_context  # persistent pools

        # ---------- persistent SBUF (whole kernel) ----------
        pers = ep(tc.tile_pool(name="pers", bufs=1))
        qt = pers.tile([128, HPC * S], MMDT, name="qt")        # Q^T, head h at [:, h*S:(h+1)*S]
        kt = pers.tile([128, S], MMDT, name="kt")              # K^T
        vn = pers.tile([128, S], MMDT, name="vn")              # V natural, tile t at [:, 128t:128t+128]
        # attn^T chunk-major: (ci, h) block at [:, ci*2048 + h*512]
        at = pers.tile([128, HPC * S], MMDT, name="at")
        ones_c = pers.tile([128, 1], MMDT, name="ones_c")
        ones_r = pers.tile([1, 128], FP32, name="ones_r")
        msk_sb = pers.tile([128, 4 * 512], MMDT, name="msk_sb")

        dram = ep(tc.tile_pool(name="dram", bufs=1, space="DRAM"))
        ga1 = dram.tile([NCORES * 128, XA1_W], MMDT, name="ga1")
        ga1_in = dram.tile([128, XA1_W], MMDT, name="ga1_in")
        ga2 = dram.tile([NCORES * 128, XA2_W], MMDT, name="ga2")
        ga2_in = dram.tile([128, XA2_W], MMDT, name="ga2_in")
        gb = dram.tile([NCORES * 128, XB_W], MMDT, name="gb")
        gb_in = dram.tile([128, XB_W], MMDT, name="gb_in")
        gat = [dram.tile([NCORES * 128, HPC * 512], MMDT, name=f"gat{ci}")
               for ci in range(NCH)]
        gat_in = [dram.tile([128, HPC * 512], MMDT, name=f"gat_in{ci}")
                  for ci in range(NCH)]

        # input AllGathers first thing; weight DMAs below overlap with them
        nc.sync.dma_start(ga1_in[:], xa1_d[:])
        nc.sync.dma_start(ga2_in[:], xa2_d[:])
        nc.sync.dma_start(gb_in[:], xcb_d[:])
        rg = [list(range(NCORES))]
        for src, dst in ((ga1_in, ga1), (ga2_in, ga2), (gb_in, gb)):
            nc.gpsimd.collective_compute(
                "AllGather", mybir.AluOpType.bypass,
                replica_groups=rg, ins=[src.opt()], outs=[dst.opt()])

        def xblk(dt_, b):
            """[128, SPC] block of X^T: d-tile dt_, seq [b*SPC, (b+1)*SPC)."""
            if dt_ < N_A1:
                g, d0 = ga1, dt_
            elif dt_ < N_A1 + N_A2:
                g, d0 = ga2, dt_ - N_A1
            else:
                g, d0 = gb, dt_ - N_A1 - N_A2
            return g[b * 128:(b + 1) * 128, d0 * SPC:(d0 + 1) * SPC]

        # ================= Phase A: projections =================
        with ExitStack() as pa:
            e = pa.enter_context
            wpool = e(tc.tile_pool(name="wpool", bufs=1))
            id_sb = wpool.tile([128, 128], MMDT, name="id_sb")
            nc.sync.dma_start(id_sb[:], id_d[:])
            csb = wpool.tile([128, S], MMDT, name="csb")
            snb = wpool.tile([128, S], MMDT, name="snb")
            cs_sb = wpool.tile([128, S], FP32, name="cs_sb")
            sn_sb = wpool.tile([128, S], FP32, name="sn_sb")
            xpool = e(tc.tile_pool(name="xpool", bufs=3))
            tpool = e(tc.tile_pool(name="tpool", bufs=2))
            psum = e(tc.tile_pool(name="psumA", bufs=1, space=bass.MemorySpace.PSUM))

            wq_t2 = [wpool.tile([128, 2 * QC], MMDT, name=f"wq2_{i}")
                     for i in range(NT_D // 2)]
            wk_t8 = [wpool.tile([128, 8 * DH], MMDT, name=f"wk8_{i}")
                     for i in range(NT_D // 8)]
            wv_t8 = [wpool.tile([128, 8 * DH], MMDT, name=f"wv8_{i}")
                     for i in range(NT_D // 8)]
            for i in range(NT_D // 2):
                nc.sync.dma_start(wq_t2[i][:], wq_d[:, i * 2 * QC:(i + 1) * 2 * QC])
            for i in range(NT_D // 8):
                nc.sync.dma_start(wk_t8[i][:], wk_d[:, i * 8 * DH:(i + 1) * 8 * DH])
                nc.sync.dma_start(wv_t8[i][:], wv_d[:, i * 8 * DH:(i + 1) * 8 * DH])
            nc.sync.dma_start(ones_c[:], on_d[:])
            nc.vector.memset(ones_r[:], 1.0)
            # unpack cos/sin/mask from the gathered xcb blocks. NOT on the
            # sync queue: these wait on the last input AG and would
            # head-of-line-block the xt tile loads behind it.
            for b in range(NCORES):
                rr = slice(b * 128, (b + 1) * 128)
                cc = slice(b * SPC, (b + 1) * SPC)
                nc.scalar.dma_start(csb[:, cc], gb[rr, CS_O:CS_O + SPC])
                nc.scalar.dma_start(snb[:, cc], gb[rr, SN_O:SN_O + SPC])
                if mode == "causal":
                    nc.scalar.dma_start(msk_sb[:, cc], gb[rr, MK_O:MK_O + SPC])
            nc.vector.tensor_copy(cs_sb[:], csb[:])
            nc.vector.tensor_copy(sn_sb[:], snb[:])

            def wq_ap(dt_, h):
                return wq_t2[dt_ // 2][:, (dt_ % 2) * QC + h * 128:
                                       (dt_ % 2) * QC + (h + 1) * 128]

            def wk_ap(dt_):
                return wk_t8[dt_ // 8][:, (dt_ % 8) * DH:(dt_ % 8 + 1) * DH]

            def wv_ap(dt_):
                return wv_t8[dt_ // 8][:, (dt_ % 8) * DH:(dt_ % 8 + 1) * DH]

            def rope_evac(src_ps, dest, ci):
                cs = cs_sb[:, ci * 512:(ci + 1) * 512]
                sn = sn_sb[:, ci * 512:(ci + 1) * 512]
                t1 = tpool.tile([128, 512], FP32, tag="t1", bufs=2)
                t2 = tpool.tile([128, 512], FP32, tag="t2", bufs=2)
                nc.vector.tensor_mul(t1[:], src_ps[:], cs)
                nc.vector.tensor_mul(t2[0:64, :], src_ps[64:128, :], sn[0:64, :])
                nc.vector.tensor_mul(t2[64:128, :], src_ps[0:64, :], sn[64:128, :])
                nc.vector.tensor_sub(dest[0:64, :], t1[0:64, :], t2[0:64, :])
                nc.vector.tensor_add(dest[64:128, :], t1[64:128, :], t2[64:128, :])

            # SBUF fp32 staging: quarter-partials accumulate here so all four
            # seq chunks consume each d-tile as soon as its AG lands (PSUM can
            # only hold one chunk's 6 accumulators at a time).
            sacc = [[wpool.tile([128, 512], FP32, name=f"sacc{ci}_{j}")
                     for j in range(6)] for ci in range(NCH)]

            def mm_group(ci, q):
                """One chunk's matmuls over quarter q into 6 fresh PSUM accs."""
                dts = QUARTERS[q]
                acc = [psum.tile([128, 512], FP32, tag="acc", bufs=6,
                                 name=f"acc{q}_{ci}_{b}") for b in range(6)]
                for i in range(dts.start // 2, dts.stop // 2):
                    xt_t = xpool.tile([128, 1024], MMDT, tag="xt", bufs=4)
                    for half in range(2):
                        dt_ = 2 * i + half
                        for k in range(2):
                            nc.sync.dma_start(
                                xt_t[:, half * 512 + k * SPC:
                                     half * 512 + (k + 1) * SPC],
                                xblk(dt_, 2 * ci + k))
                    for half in range(2):
                        dt_ = 2 * i + half
                        st = dt_ == dts.start
                        sp = dt_ == dts.stop - 1
                        rhs = xt_t[:, half * 512:(half + 1) * 512]
                        for h in range(HPC):
                            nc.tensor.matmul(acc[h][:], wq_ap(dt_, h), rhs,
                                             start=st, stop=sp)
                        nc.tensor.matmul(acc[4][:], wk_ap(dt_), rhs,
                                         start=st, stop=sp)
                        nc.tensor.matmul(acc[5][:], wv_ap(dt_), rhs,
                                         start=st, stop=sp)
                return acc

            pend_tr = []   # deferred V transposes: emitted one chunk late so
                           # the PE never waits on the DVE that produces vt_t

            def flush_tr():
                for vt_t, ci in pend_tr:
                    for i in range(4):
                        ps_tr = psum.tile([128, 128], MMDT, tag="tr", bufs=2,
                                          name=f"tr{ci}_{i}")
                        nc.tensor.transpose(ps_tr[:], vt_t[:, i * 128:(i + 1) * 128],
                                            id_sb[:])
                        s0 = (ci * 4 + i) * 128
                        nc.vector.tensor_copy(vn[:, s0:s0 + 128], ps_tr[:])
                pend_tr.clear()

            for q in range(4):
                for ci in range(NCH):
                    acc = mm_group(ci, q)
                    if q == 0:
                        for j in range(6):
                            nc.vector.tensor_copy(sacc[ci][j][:], acc[j][:])
                    elif q < 3:
                        for j in range(6):
                            nc.vector.tensor_add(sacc[ci][j][:], sacc[ci][j][:],
                                                 acc[j][:])
                    else:
                        # final quarter: fold the SBUF partial INTO PSUM and
                        # evacuate from there (the rope partition-half shuffle
                        # needs one PSUM operand - two SBUF inputs with
                        # different base partitions are illegal on DVE).
                        # V first (gates the PE transposes), then Q/K rope.
                        nc.vector.tensor_add(acc[5][:], acc[5][:], sacc[ci][5][:])
                        vt_t = tpool.tile([128, 512], MMDT, tag="vt", bufs=2)
                        nc.scalar.copy(vt_t[:], acc[5][:])
                        flush_tr()
                        pend_tr.append((vt_t, ci))
                        for h in range(HPC):
                            nc.vector.tensor_add(acc[h][:], acc[h][:],
                                                 sacc[ci][h][:])
                            rope_evac(acc[h],
                                      qt[:, h * S + ci * 512:h * S + (ci + 1) * 512],
                                      ci)
                        nc.vector.tensor_add(acc[4][:], acc[4][:],
                                             sacc[ci][4][:])
                        rope_evac(acc[4], kt[:, ci * 512:(ci + 1) * 512], ci)
            flush_tr()

        if "B" not in phases:
            return

        # ================= Phase B: attention =================
        # at chunk-major; each finished chunk is AllGathered while later
        # chunks are still computing.
        with ExitStack() as pb:
            e = pb.enter_context
            ppool = e(tc.tile_pool(name="ppool", bufs=4))
            npool = e(tc.tile_pool(name="npool", bufs=2))
            mpool = e(tc.tile_pool(name="mpool", bufs=4))
            psum = e(tc.tile_pool(name="psumB", bufs=1, space=bass.MemorySpace.PSUM))

            for ci in range(NCH):
                n_sk = 4 * (ci + 1) if mode == "causal" else S // 128
                for h in range(HPC):
                    ps_pv = psum.tile([128, 512], FP32, tag="pv", bufs=2,
                                      name=f"pv{ci}_{h}")
                    ps_sm = psum.tile([1, 512], FP32, tag="sm", bufs=2,
                                      name=f"sm{ci}_{h}")
                    qs = qt[:, h * S + ci * 512:h * S + (ci + 1) * 512]
                    for sk in range(n_sk):
                        ps_sc = psum.tile([128, 512], FP32, tag="sc", bufs=2,
                                          name=f"sc{ci}_{h}_{sk}")
                        nc.tensor.matmul(ps_sc[:], kt[:, sk * 128:(sk + 1) * 128],
                                         qs, start=True, stop=True)
                        p = ppool.tile([128, 512], MMDT, tag="p", bufs=4)
                        if mode == "masked":
                            mt = mpool.tile([128, 512], FP32, tag="mt", bufs=4)
                            nc.sync.dma_start(
                                mt[:], msk_d[sk * 128:(sk + 1) * 128,
                                             ci * 512:(ci + 1) * 512])
                            nc.vector.tensor_scalar_mul(p[:], ps_sc[:], SCALE)
                            nc.vector.tensor_add(p[:], p[:], mt[:])
                            nc.scalar.activation(p[:], p[:], AF.Exp)
                        else:
                            nc.scalar.activation(p[:], ps_sc[:], AF.Exp, scale=SCALE)
                            if mode == "causal" and sk >= 4 * ci:
                                j = sk - 4 * ci
                                nc.vector.tensor_mul(
                                    p[:], p[:], msk_sb[:, j * 512:(j + 1) * 512])
                        st = sk == 0
                        sp = sk == n_sk - 1
                        nc.tensor.matmul(ps_pv[:], vn[:, sk * 128:(sk + 1) * 128],
                                         p[:], start=st, stop=sp)
                        nc.tensor.matmul(ps_sm[:], ones_c[:], p[:],
                                         start=st, stop=sp)
                    # normalize: 1/sums broadcast over partitions via K=1 matmul
                    rc = npool.tile([1, 512], FP32, tag="rc", bufs=2)
                    rs = npool.tile([1, 512], FP32, tag="rs", bufs=2)
                    nc.vector.reciprocal_approx_accurate(rc[:], ps_sm[:], rs[:])
                    ps_bc = psum.tile([128, 512], FP32, tag="bc", bufs=2,
                                      name=f"bc{ci}_{h}")
                    nc.tensor.matmul(ps_bc[:], ones_r[:], rc[:], start=True, stop=True)
                    rb = npool.tile([128, 512], FP32, tag="rb", bufs=2)
                    nc.scalar.copy(rb[:], ps_bc[:])
                    nc.vector.tensor_mul(at[:, ci * 2048 + h * 512:
                                            ci * 2048 + (h + 1) * 512],
                                         ps_pv[:], rb[:])
                # ship this chunk's attn^T while later chunks compute
                nc.sync.dma_start(gat_in[ci][:], at[:, ci * 2048:(ci + 1) * 2048])
                nc.gpsimd.collective_compute(
                    "AllGather", mybir.AluOpType.bypass,
                    replica_groups=rg, ins=[gat_in[ci].opt()],
                    outs=[gat[ci].opt()])

        if "C" not in phases:
            return
        # ====== Phase C: project gathered heads into this core's 512 columns ====
        with ExitStack() as pc:
            e = pc.enter_context
            wopool = e(tc.tile_pool(name="wopool", bufs=1))
            apool = e(tc.tile_pool(name="apool", bufs=3))
            opool = e(tc.tile_pool(name="opool", bufs=4))
            psum = e(tc.tile_pool(name="psumC", bufs=1, space=bass.MemorySpace.PSUM))
            wo_sb = wopool.tile([128, H * 512], MMDT, name="wo_sb")
            for i in range(4):
                nc.sync.dma_start(wo_sb[:, i * 8 * 512:(i + 1) * 8 * 512],
                                  wo_d[:, i * 8 * 512:(i + 1) * 8 * 512])
            for ci in range(NCH):
                ps_o = [psum.tile([128, 512], FP32, tag="oo", bufs=8,
                                  name=f"oo{ci}_{j}") for j in range(4)]
                for b in range(NCORES):
                    a2 = apool.tile([128, HPC * 512], MMDT, tag="a2", bufs=3)
                    nc.sync.dma_start(a2[:], gat[ci][b * 128:(b + 1) * 128, :])
                    for sub in range(HPC):
                        hh = b * HPC + sub
                        for j in range(4):
                            nc.tensor.matmul(
                                ps_o[j][:],
                                a2[:, sub * 512 + j * 128:sub * 512 + (j + 1) * 128],
                                wo_sb[:, hh * 512:(hh + 1) * 512],
                                start=(b == 0 and sub == 0),
                                stop=(b == NCORES - 1 and sub == HPC - 1))
                for j in range(4):
                    sb = ci * 4 + j
                    ob = opool.tile([128, 512], MMDT, tag="ob", bufs=4)
                    nc.vector.tensor_copy(ob[:], ps_o[j][:])
                    nc.sync.dma_start(out_d[sb * 128:(sb + 1) * 128, :], ob[:])


def build(mode="causal", phases="ABC"):
    nc = bacc.Bacc("TRN2", target_bir_lowering=False, debug=False,
                   num_devices=NCORES)
    xa1_d = nc.dram_tensor("xa1", [128, XA1_W], MMDT, kind="ExternalInput").ap()
    xa2_d = nc.dram_tensor("xa2", [128, XA2_W], MMDT, kind="ExternalInput").ap()
    xcb_d = nc.dram_tensor("xcb", [128, XB_W], MMDT, kind="ExternalInput").ap()
    wq_d = nc.dram_tensor("wq", [128, NT_D * QC], MMDT, kind="ExternalInput").ap()
    wk_d = nc.dram_tensor("wk", [128, NT_D * DH], MMDT, kind="ExternalInput").ap()
    wv_d = nc.dram_tensor("wv", [128, NT_D * DH], MMDT, kind="ExternalInput").ap()
    wo_d = nc.dram_tensor("wo", [128, H * 512], MMDT, kind="ExternalInput").ap()
    # masked: [S, S] additive mask^T streamed from DRAM (otherwise unused dummy)
    mshape2 = [S, S] if mode == "masked" else [1, 1]
    msk_d = nc.dram_tensor("msk", mshape2, FP32, kind="ExternalInput").ap()
    id_d = nc.dram_tensor("ident", [128, 128], MMDT, kind="ExternalInput").ap()
    on_d = nc.dram_tensor("ones", [128, 1], MMDT, kind="ExternalInput").ap()
    out_d = nc.dram_tensor("out", [S, QC], MMDT, kind="ExternalOutput").ap()
    io = (xa1_d, xa2_d, xcb_d, wq_d, wk_d, wv_d, wo_d, msk_d, id_d, on_d, out_d)
    with tile.TileContext(nc) as tc:
        _emit(nc, tc, io, mode, phases)
    nc.compile()
    return nc


_CACHE = {}
RUN_KWARGS = {}   # extra kwargs for run_bass_kernel_spmd (e.g. trace=True)
LAST = None       # last BassKernelResults (for exec_time_ns inspection)

_CAUSAL_REF = None


def _causal_ref_mask():
    global _CAUSAL_REF
    if _CAUSAL_REF is None:
        neg = np.finfo(np.float32).min
        m = np.where(np.tril(np.ones((S, S), dtype=bool)), 0.0, neg)
        _CAUSAL_REF = m.astype(np.float32)
    return _CAUSAL_REF


def _tile_rows(w):
    # [T*128, C] -> [128, T*C] with d-tile blocks along free dim
    t = w.shape[0] // 128
    return np.ascontiguousarray(
        w.reshape(t, 128, w.shape[1]).transpose(1, 0, 2).reshape(128, -1))


def make_in_maps(hidden_states, cos, sin, attention_mask, Wq, Wk, Wv, Wo, mode):
    mdt = _np_mmdt()
    xtb = np.asarray(hidden_states).reshape(S, D).T.astype(mdt)   # [4096, 2048]
    xblk = xtb.reshape(NT_D, 128, S)                              # [32, 128, 2048]
    cosT = np.asarray(cos).T.astype(mdt)                          # [128, 2048]
    sinT = np.asarray(sin).T.astype(mdt)
    ident = np.eye(128, dtype=mdt)
    if mode == "masked":
        msk = np.ascontiguousarray(
            np.asarray(attention_mask).reshape(S, S).T).astype(np.float32)
    else:
        msk = np.zeros((1, 1), dtype=np.float32)
    if mode == "causal":
        # 4 diagonal 0/1 tiles: tile j valid where 128*j + k <= q  (k:[128], q:[512])
        j = np.arange(4)[:, None, None]
        k = np.arange(128)[None, :, None]
        q = np.arange(512)[None, None, :]
        mflat = np.ascontiguousarray((128 * j + k <= q).astype(mdt)
                                     .transpose(1, 0, 2).reshape(128, 4 * 512))
    else:
        mflat = np.zeros((128, 4 * 512), dtype=mdt)
    ones = np.ones((128, 1), dtype=mdt)
    in_maps = []
    for c in range(NCORES):
        cc = slice(c * SPC, (c + 1) * SPC)
        xa1 = np.ascontiguousarray(
            xblk[:N_A1, :, cc].transpose(1, 0, 2).reshape(128, XA1_W))
        xa2 = np.ascontiguousarray(
            xblk[N_A1:N_A1 + N_A2, :, cc].transpose(1, 0, 2).reshape(128, XA2_W))
        xb = np.empty((128, XB_W), dtype=mdt)
        xb[:, :XB_X] = (xblk[N_A1 + N_A2:, :, cc]
                        .transpose(1, 0, 2).reshape(128, XB_X))
        xb[:, CS_O:CS_O + SPC] = cosT[:, cc]
        xb[:, SN_O:SN_O + SPC] = sinT[:, cc]
        xb[:, MK_O:MK_O + SPC] = mflat[:, cc]
        in_maps.append({
            "xa1": xa1, "xa2": xa2, "xcb": xb,
            "wq": _tile_rows(np.asarray(Wq[:, c * QC:(c + 1) * QC]).astype(mdt)),
            "wk": _tile_rows(np.asarray(Wk[:, c * DH:(c + 1) * DH]).astype(mdt)),
            "wv": _tile_rows(np.asarray(Wv[:, c * DH:(c + 1) * DH]).astype(mdt)),
            "wo": _tile_rows(np.asarray(Wo[:, c * QC:(c + 1) * QC]).astype(mdt)),
            "msk": msk, "ident": ident, "ones": ones,
        })
    return in_maps


def pick_mode(attention_mask):
    am = np.asarray(attention_mask).reshape(S, S)
    if np.array_equal(am, _causal_ref_mask()):
        return "causal"
    if not np.any(am):
        return "dense"
    return "masked"


def kernel(hidden_states, cos, sin, attention_mask, Wq, Wk, Wv, Wo, **kwargs):
    mode = pick_mode(attention_mask)
    ck = (mode, str(MMDT))
    if ck not in _CACHE:
        _CACHE[ck] = build(mode)
    nc = _CACHE[ck]
    in_maps = make_in_maps(hidden_states, cos, sin, attention_mask,
                           Wq, Wk, Wv, Wo, mode)
    res = run_bass_kernel_spmd(nc, in_maps, core_ids=list(range(NCORES)),
                               **RUN_KWARGS)
    global LAST
    LAST = res
    out = np.concatenate([res.results[c]["out"] for c in range(NCORES)], axis=1)
    return out.astype(np.float32).reshape(1, S, D)


# revision 32
# speedup vs baseline: 1.0083x; 1.0083x over previous
"""Llama GQA attention layer (B=1, S=2048, D=4096, H=32, KVH=8, DH=128) on 8 trn2 cores.

Sharding: tensor-parallel over heads. Core c owns Q heads [4c, 4c+4) and KV head c:
  Wq[:, c*512:(c+1)*512], Wk/Wv[:, c*128:(c+1)*128], Wo columns [c*512:(c+1)*512].

Data logistics are the dominant cost on this axon-tunneled setup (~55 MB/s to the
device), so the kernel minimizes host<->device bytes:
  - hidden_states / cos / sin / mask are SEQUENCE-SHARDED on the host (each core
    receives its 256-seq slice, bf16, split into three packed tensors) and
    AllGathered on device (three pipelined AGs so projections start after the
    first small one).
  - Wo is sharded COLUMN-wise; after attention, the per-head attention outputs
    (at, bf16) are AllGathered on device CHUNK-WISE (4 AGs overlapping phase B)
    so each core computes a DISJOINT [2048, 512] output slice. No host-side
    reduction, no fp32 partials: the output download is 8 x 2 MB bf16 instead
    of 8 x 32 MB fp32 (and the donated zero output buffers shrink identically).

Kernel layout strategy (per core):
  - X^T [4096, 2048] from the gathered blocks; projections computed as
    Q^T/K^T/V^T [dh, s] via PSUM accumulation over 32 d-tiles.
  - RoPE applied on PSUM evacuation (DVE, partition-half shuffle).
  - V^T transposed to V natural [s, dh] via PE-transpose (needed as PV stationary).
  - Attention with scores TRANSPOSED: S^T[k, q] tiles [128, 512] so softmax sums
    over keys become ones-vector matmuls; exp on ACT (no max subtraction - scores
    are O(10), exp is safe); causal sparsity by skipping fully-masked key tiles;
    diagonal tiles masked multiplicatively with 4 static 0/1 tiles.
  - Softmax normalization: recip of sums row [1,512] broadcast across partitions
    via a K=1 ones matmul, then one DVE mul per attn^T tile.
  - at stored chunk-major ([128, ci*2048 + h*512 + q]) so each chunk's AG input
    is one contiguous [128, 2048] slice; phase C walks (chunk, rank-block) with
    one 0.5 MB DMA per block and accumulates all 32 heads into 4 PSUM banks.
"""

import os as _os

import numpy as np

import concourse.bass as bass
import concourse.bacc as bacc
import concourse.mybir as mybir
import concourse.tile as tile
from concourse.bass_utils import run_bass_kernel_spmd

S = 2048
D = 4096
H = 32
KVH = 8
DH = 128
NCORES = 8
HPC = H // NCORES            # 4 query heads per core
QC = HPC * DH                # 512 projection cols per core
SCALE = float(DH) ** -0.5
NT_D = D // 128              # 32 contraction tiles
NCH = S // 512               # 4 sequence chunks
SPC = S // NCORES            # 256 seq positions shipped per core
FP32 = mybir.dt.float32
FP32R = mybir.dt.float32r
BF16 = mybir.dt.bfloat16
AF = mybir.ActivationFunctionType

MMDT = {"bf16": BF16, "fp32r": FP32R}[_os.environ.get("KERNEL_MM_DTYPE", "bf16")]

# per-core input split in four packed tensors (four pipelined AllGathers;
# phase A walks d-tiles in AG order across ALL four seq chunks, so matmuls
# start right after the first small AG and never starve on the later ones):
#   xa1: d-tiles 0..3                       [128,  4*SPC] = [128, 1024]
#   xa2: d-tiles 4..11                      [128,  8*SPC] = [128, 2048]
#   xb1: d-tiles 12..21                     [128, 10*SPC] = [128, 2560]
#   xb2: d-tiles 22..31 | cos | sin | mask  [128, 13*SPC] = [128, 3328]
N_SPLIT = (4, 8, 10, 10)     # d-tiles per AG, in arrival order
XA1_W = N_SPLIT[0] * SPC     # 1024
XA2_W = N_SPLIT[1] * SPC     # 2048
XB1_W = N_SPLIT[2] * SPC     # 2560
XB_X = N_SPLIT[3] * SPC      # 2560 (xt portion of xb2)
CS_O = XB_X                  # cos offset inside xb2's tail
SN_O = XB_X + SPC
MK_O = XB_X + 2 * SPC
XB_W = XB_X + 3 * SPC        # 3328
_B = [0, 4, 12, 22, 32]      # d-tile split boundaries
# d-tile quarters matching AG arrival order (each a run of dt-pairs)
QUARTERS = [range(_B[i], _B[i + 1]) for i in range(4)]


def _np_mmdt():
    import ml_dtypes
    return {BF16: ml_dtypes.bfloat16, FP32R: np.float32}[MMDT]


def _emit(nc, tc, io, mode, phases="ABC"):
    """mode: 'causal' (sparse, static diag masks), 'dense' (all tiles, no mask),
    'masked' (all tiles, additive mask streamed from DRAM)."""
    from contextlib import ExitStack

    xa1_d, xa2_d, xb1_d, xb2_d, wq_d, wk_d, wv_d, wo_d, msk_d, id_d, on_d, out_d = io

    with ExitStack() as top:
        ep = top.enter_context  # persistent pools

        # ---------- persistent SBUF (whole kernel) ----------
        pers = ep(tc.tile_pool(name="pers", bufs=1))
        qt = pers.tile([128, HPC * S], MMDT, name="qt")        # Q^T, head h at [:, h*S:(h+1)*S]
        kt = pers.tile([128, S], MMDT, name="kt")              # K^T
        vn = pers.tile([128, S], MMDT, name="vn")              # V natural, tile t at [:, 128t:128t+128]
        # attn^T chunk-major: (ci, h) block at [:, ci*2048 + h*512]
        at = pers.tile([128, HPC * S], MMDT, name="at")
        ones_c = pers.tile([128, 1], MMDT, name="ones_c")
        ones_r = pers.tile([1, 128], FP32, name="ones_r")
        msk_sb = pers.tile([128, 4 * 512], MMDT, name="msk_sb")

        dram = ep(tc.tile_pool(name="dram", bufs=1, space="DRAM"))
        widths = (XA1_W, XA2_W, XB1_W, XB_W)
        gx = [dram.tile([NCORES * 128, w], MMDT, name=f"gx{i}")
              for i, w in enumerate(widths)]
        gx_in = [dram.tile([128, w], MMDT, name=f"gx_in{i}")
                 for i, w in enumerate(widths)]
        gb = gx[3]
        gat = [dram.tile([NCORES * 128, HPC * 512], MMDT, name=f"gat{ci}")
               for ci in range(NCH)]
        gat_in = [dram.tile([128, HPC * 512], MMDT, name=f"gat_in{ci}")
                  for ci in range(NCH)]

        # input AllGathers first thing; weight DMAs below overlap with them
        for src, d_in in zip(gx_in, (xa1_d, xa2_d, xb1_d, xb2_d)):
            nc.sync.dma_start(src[:], d_in[:])
        rg = [list(range(NCORES))]
        for src, dst in zip(gx_in, gx):
            nc.gpsimd.collective_compute(
                "AllGather", mybir.AluOpType.bypass,
                replica_groups=rg, ins=[src.opt()], outs=[dst.opt()])

        def xblk(dt_, b):
            """[128, SPC] block of X^T: d-tile dt_, seq [b*SPC, (b+1)*SPC)."""
            for i in range(4):
                if dt_ < _B[i + 1]:
                    g, d0 = gx[i], dt_ - _B[i]
                    break
            return g[b * 128:(b + 1) * 128, d0 * SPC:(d0 + 1) * SPC]

        # ================= Phase A: projections =================
        with ExitStack() as pa:
            e = pa.enter_context
            wpool = e(tc.tile_pool(name="wpool", bufs=1))
            id_sb = wpool.tile([128, 128], MMDT, name="id_sb")
            nc.sync.dma_start(id_sb[:], id_d[:])
            csb = wpool.tile([128, S], MMDT, name="csb")
            snb = wpool.tile([128, S], MMDT, name="snb")
            cs_sb = wpool.tile([128, S], FP32, name="cs_sb")
            sn_sb = wpool.tile([128, S], FP32, name="sn_sb")
            xpool = e(tc.tile_pool(name="xpool", bufs=3))
            tpool = e(tc.tile_pool(name="tpool", bufs=2))
            psum = e(tc.tile_pool(name="psumA", bufs=1, space=bass.MemorySpace.PSUM))

            wq_t2 = [wpool.tile([128, 2 * QC], MMDT, name=f"wq2_{i}")
                     for i in range(NT_D // 2)]
            wk_t8 = [wpool.tile([128, 8 * DH], MMDT, name=f"wk8_{i}")
                     for i in range(NT_D // 8)]
            wv_t8 = [wpool.tile([128, 8 * DH], MMDT, name=f"wv8_{i}")
                     for i in range(NT_D // 8)]
            for i in range(NT_D // 2):
                nc.sync.dma_start(wq_t2[i][:], wq_d[:, i * 2 * QC:(i + 1) * 2 * QC])
            for i in range(NT_D // 8):
                nc.sync.dma_start(wk_t8[i][:], wk_d[:, i * 8 * DH:(i + 1) * 8 * DH])
                nc.sync.dma_start(wv_t8[i][:], wv_d[:, i * 8 * DH:(i + 1) * 8 * DH])
            nc.sync.dma_start(ones_c[:], on_d[:])
            nc.vector.memset(ones_r[:], 1.0)
            # unpack cos/sin/mask from the gathered xcb blocks. NOT on the
            # sync queue: these wait on the last input AG and would
            # head-of-line-block the xt tile loads behind it.
            for b in range(NCORES):
                rr = slice(b * 128, (b + 1) * 128)
                cc = slice(b * SPC, (b + 1) * SPC)
                nc.scalar.dma_start(csb[:, cc], gb[rr, CS_O:CS_O + SPC])
                nc.scalar.dma_start(snb[:, cc], gb[rr, SN_O:SN_O + SPC])
                if mode == "causal":
                    nc.scalar.dma_start(msk_sb[:, cc], gb[rr, MK_O:MK_O + SPC])
            nc.vector.tensor_copy(cs_sb[:], csb[:])
            nc.vector.tensor_copy(sn_sb[:], snb[:])

            def wq_ap(dt_, h):
                return wq_t2[dt_ // 2][:, (dt_ % 2) * QC + h * 128:
                                       (dt_ % 2) * QC + (h + 1) * 128]

            def wk_ap(dt_):
                return wk_t8[dt_ // 8][:, (dt_ % 8) * DH:(dt_ % 8 + 1) * DH]

            def wv_ap(dt_):
                return wv_t8[dt_ // 8][:, (dt_ % 8) * DH:(dt_ % 8 + 1) * DH]

            def rope_evac(src_ps, dest, ci):
                cs = cs_sb[:, ci * 512:(ci + 1) * 512]
                sn = sn_sb[:, ci * 512:(ci + 1) * 512]
                t1 = tpool.tile([128, 512], FP32, tag="t1", bufs=2)
                t2 = tpool.tile([128, 512], FP32, tag="t2", bufs=2)
                nc.vector.tensor_mul(t1[:], src_ps[:], cs)
                nc.vector.tensor_mul(t2[0:64, :], src_ps[64:128, :], sn[0:64, :])
                nc.vector.tensor_mul(t2[64:128, :], src_ps[0:64, :], sn[64:128, :])
                nc.vector.tensor_sub(dest[0:64, :], t1[0:64, :], t2[0:64, :])
                nc.vector.tensor_add(dest[64:128, :], t1[64:128, :], t2[64:128, :])

            # SBUF fp32 staging: quarter-partials accumulate here so all four
            # seq chunks consume each d-tile as soon as its AG lands (PSUM can
            # only hold one chunk's 6 accumulators at a time).
            sacc = [[wpool.tile([128, 512], FP32, name=f"sacc{ci}_{j}")
                     for j in range(6)] for ci in range(NCH)]

            def mm_group(ci, q):
                """One chunk's matmuls over quarter q into 6 fresh PSUM accs."""
                dts = QUARTERS[q]
                acc = [psum.tile([128, 512], FP32, tag="acc", bufs=6,
                                 name=f"acc{q}_{ci}_{b}") for b in range(6)]
                for i in range(dts.start // 2, dts.stop // 2):
                    xt_t = xpool.tile([128, 1024], MMDT, tag="xt", bufs=4)
                    for half in range(2):
                        dt_ = 2 * i + half
                        for k in range(2):
                            nc.sync.dma_start(
                                xt_t[:, half * 512 + k * SPC:
                                     half * 512 + (k + 1) * SPC],
                                xblk(dt_, 2 * ci + k))
                    for half in range(2):
                        dt_ = 2 * i + half
                        st = dt_ == dts.start
                        sp = dt_ == dts.stop - 1
                        rhs = xt_t[:, half * 512:(half + 1) * 512]
                        for h in range(HPC):
                            nc.tensor.matmul(acc[h][:], wq_ap(dt_, h), rhs,
                                             start=st, stop=sp)
                        nc.tensor.matmul(acc[4][:], wk_ap(dt_), rhs,
                                         start=st, stop=sp)
                        nc.tensor.matmul(acc[5][:], wv_ap(dt_), rhs,
                                         start=st, stop=sp)
                return acc

            pend_tr = []   # deferred V transposes: emitted one chunk late so
                           # the PE never waits on the DVE that produces vt_t

            def flush_tr():
                for vt_t, ci in pend_tr:
                    for i in range(4):
                        ps_tr = psum.tile([128, 128], MMDT, tag="tr", bufs=2,
                                          name=f"tr{ci}_{i}")
                        nc.tensor.transpose(ps_tr[:], vt_t[:, i * 128:(i + 1) * 128],
                                            id_sb[:])
                        s0 = (ci * 4 + i) * 128
                        nc.vector.tensor_copy(vn[:, s0:s0 + 128], ps_tr[:])
                pend_tr.clear()

            for q in range(4):
                for ci in range(NCH):
                    acc = mm_group(ci, q)
                    if q == 0:
                        for j in range(6):
                            nc.vector.tensor_copy(sacc[ci][j][:], acc[j][:])
                    elif q < 3:
                        for j in range(6):
                            nc.vector.tensor_add(sacc[ci][j][:], sacc[ci][j][:],
                                                 acc[j][:])
                    else:
                        # final quarter: fold the SBUF partial INTO PSUM and
                        # evacuate from there (the rope partition-half shuffle
                        # needs one PSUM operand - two SBUF inputs with
                        # different base partitions are illegal on DVE).
                        # V first (gates the PE transposes), then Q/K rope.
                        nc.vector.tensor_add(acc[5][:], acc[5][:], sacc[ci][5][:])
                        vt_t = tpool.tile([128, 512], MMDT, tag="vt", bufs=2)
                        nc.scalar.copy(vt_t[:], acc[5][:])
                        flush_tr()
                        pend_tr.append((vt_t, ci))
                        for h in range(HPC):
                            nc.vector.tensor_add(acc[h][:], acc[h][:],
                                                 sacc[ci][h][:])
                            rope_evac(acc[h],
                                      qt[:, h * S + ci * 512:h * S + (ci + 1) * 512],
                                      ci)
                        nc.vector.tensor_add(acc[4][:], acc[4][:],
                                             sacc[ci][4][:])
                        rope_evac(acc[4], kt[:, ci * 512:(ci + 1) * 512], ci)
            flush_tr()

        if "B" not in phases:
            return

        # ================= Phase B: attention =================
        # at chunk-major; each finished chunk is AllGathered while later
        # chunks are still computing.
        with ExitStack() as pb:
            e = pb.enter_context
            ppool = e(tc.tile_pool(name="ppool", bufs=4))
            npool = e(tc.tile_pool(name="npool", bufs=2))
            mpool = e(tc.tile_pool(name="mpool", bufs=4))
            psum = e(tc.tile_pool(name="psumB", bufs=1, space=bass.MemorySpace.PSUM))

            for ci in range(NCH):
                n_sk = 4 * (ci + 1) if mode == "causal" else S // 128
                for h in range(HPC):
                    ps_pv = psum.tile([128, 512], FP32, tag="pv", bufs=2,
                                      name=f"pv{ci}_{h}")
                    ps_sm = psum.tile([1, 512], FP32, tag="sm", bufs=2,
                                      name=f"sm{ci}_{h}")
                    qs = qt[:, h * S + ci * 512:h * S + (ci + 1) * 512]
                    for sk in range(n_sk):
                        ps_sc = psum.tile([128, 512], FP32, tag="sc", bufs=2,
                                          name=f"sc{ci}_{h}_{sk}")
                        nc.tensor.matmul(ps_sc[:], kt[:, sk * 128:(sk + 1) * 128],
                                         qs, start=True, stop=True)
                        p = ppool.tile([128, 512], MMDT, tag="p", bufs=4)
                        if mode == "masked":
                            mt = mpool.tile([128, 512], FP32, tag="mt", bufs=4)
                            nc.sync.dma_start(
                                mt[:], msk_d[sk * 128:(sk + 1) * 128,
                                             ci * 512:(ci + 1) * 512])
                            nc.vector.tensor_scalar_mul(p[:], ps_sc[:], SCALE)
                            nc.vector.tensor_add(p[:], p[:], mt[:])
                            nc.scalar.activation(p[:], p[:], AF.Exp)
                        else:
                            nc.scalar.activation(p[:], ps_sc[:], AF.Exp, scale=SCALE)
                            if mode == "causal" and sk >= 4 * ci:
                                j = sk - 4 * ci
                                nc.vector.tensor_mul(
                                    p[:], p[:], msk_sb[:, j * 512:(j + 1) * 512])
                        st = sk == 0
                        sp = sk == n_sk - 1
                        nc.tensor.matmul(ps_pv[:], vn[:, sk * 128:(sk + 1) * 128],
                                         p[:], start=st, stop=sp)
                        nc.tensor.matmul(ps_sm[:], ones_c[:], p[:],
                                         start=st, stop=sp)
                    # normalize: 1/sums broadcast over partitions via K=1 matmul
                    rc = npool.tile([1, 512], FP32, tag="rc", bufs=2)
                    rs = npool.tile([1, 512], FP32, tag="rs", bufs=2)
                    nc.vector.reciprocal_approx_accurate(rc[:], ps_sm[:], rs[:])
                    ps_bc = psum.tile([128, 512], FP32, tag="bc", bufs=2,
                                      name=f"bc{ci}_{h}")
                    nc.tensor.matmul(ps_bc[:], ones_r[:], rc[:], start=True, stop=True)
                    rb = npool.tile([128, 512], FP32, tag="rb", bufs=2)
                    nc.scalar.copy(rb[:], ps_bc[:])
                    nc.vector.tensor_mul(at[:, ci * 2048 + h * 512:
                                            ci * 2048 + (h + 1) * 512],
                                         ps_pv[:], rb[:])
                # ship this chunk's attn^T while later chunks compute
                nc.sync.dma_start(gat_in[ci][:], at[:, ci * 2048:(ci + 1) * 2048])
                nc.gpsimd.collective_compute(
                    "AllGather", mybir.AluOpType.bypass,
                    replica_groups=rg, ins=[gat_in[ci].opt()],
                    outs=[gat[ci].opt()])

        if "C" not in phases:
            return
        # ====== Phase C: project gathered heads into this core's 512 columns ====
        with ExitStack() as pc:
            e = pc.enter_context
            wopool = e(tc.tile_pool(name="wopool", bufs=1))
            apool = e(tc.tile_pool(name="apool", bufs=3))
            opool = e(tc.tile_pool(name="opool", bufs=4))
            psum = e(tc.tile_pool(name="psumC", bufs=1, space=bass.MemorySpace.PSUM))
            wo_sb = wopool.tile([128, H * 512], MMDT, name="wo_sb")
            for i in range(4):
                nc.sync.dma_start(wo_sb[:, i * 8 * 512:(i + 1) * 8 * 512],
                                  wo_d[:, i * 8 * 512:(i + 1) * 8 * 512])
            for ci in range(NCH):
                ps_o = [psum.tile([128, 512], FP32, tag="oo", bufs=8,
                                  name=f"oo{ci}_{j}") for j in range(4)]
                for b in range(NCORES):
                    a2 = apool.tile([128, HPC * 512], MMDT, tag="a2", bufs=3)
                    nc.sync.dma_start(a2[:], gat[ci][b * 128:(b + 1) * 128, :])
                    for sub in range(HPC):
                        hh = b * HPC + sub
                        for j in range(4):
                            nc.tensor.matmul(
                                ps_o[j][:],
                                a2[:, sub * 512 + j * 128:sub * 512 + (j + 1) * 128],
                                wo_sb[:, hh * 512:(hh + 1) * 512],
                                start=(b == 0 and sub == 0),
                                stop=(b == NCORES - 1 and sub == HPC - 1))
                for j in range(4):
                    sb = ci * 4 + j
                    ob = opool.tile([128, 512], MMDT, tag="ob", bufs=4)
                    nc.vector.tensor_copy(ob[:], ps_o[j][:])
                    nc.sync.dma_start(out_d[sb * 128:(sb + 1) * 128, :], ob[:])


def build(mode="causal", phases="ABC"):
    nc = bacc.Bacc("TRN2", target_bir_lowering=False, debug=False,
                   num_devices=NCORES)
    xa1_d = nc.dram_tensor("xa1", [128, XA1_W], MMDT, kind="ExternalInput").ap()
    xa2_d = nc.dram_tensor("xa2", [128, XA2_W], MMDT, kind="ExternalInput").ap()
    xb1_d = nc.dram_tensor("xb1", [128, XB1_W], MMDT, kind="ExternalInput").ap()
    xb2_d = nc.dram_tensor("xb2", [128, XB_W], MMDT, kind="ExternalInput").ap()
    wq_d = nc.dram_tensor("wq", [128, NT_D * QC], MMDT, kind="ExternalInput").ap()
    wk_d = nc.dram_tensor("wk", [128, NT_D * DH], MMDT, kind="ExternalInput").ap()
    wv_d = nc.dram_tensor("wv", [128, NT_D * DH], MMDT, kind="ExternalInput").ap()
    wo_d = nc.dram_tensor("wo", [128, H * 512], MMDT, kind="ExternalInput").ap()
    # masked: [S, S] additive mask^T streamed from DRAM (otherwise unused dummy)
    mshape2 = [S, S] if mode == "masked" else [1, 1]
    msk_d = nc.dram_tensor("msk", mshape2, FP32, kind="ExternalInput").ap()
    id_d = nc.dram_tensor("ident", [128, 128], MMDT, kind="ExternalInput").ap()
    on_d = nc.dram_tensor("ones", [128, 1], MMDT, kind="ExternalInput").ap()
    out_d = nc.dram_tensor("out", [S, QC], MMDT, kind="ExternalOutput").ap()
    io = (xa1_d, xa2_d, xb1_d, xb2_d, wq_d, wk_d, wv_d, wo_d, msk_d, id_d,
          on_d, out_d)
    with tile.TileContext(nc) as tc:
        _emit(nc, tc, io, mode, phases)
    nc.compile()
    return nc


_CACHE = {}
RUN_KWARGS = {}   # extra kwargs for run_bass_kernel_spmd (e.g. trace=True)
LAST = None       # last BassKernelResults (for exec_time_ns inspection)

_CAUSAL_REF = None


def _causal_ref_mask():
    global _CAUSAL_REF
    if _CAUSAL_REF is None:
        neg = np.finfo(np.float32).min
        m = np.where(np.tril(np.ones((S, S), dtype=bool)), 0.0, neg)
        _CAUSAL_REF = m.astype(np.float32)
    return _CAUSAL_REF


def _tile_rows(w):
    # [T*128, C] -> [128, T*C] with d-tile blocks along free dim
    t = w.shape[0] // 128
    return np.ascontiguousarray(
        w.reshape(t, 128, w.shape[1]).transpose(1, 0, 2).reshape(128, -1))


def make_in_maps(hidden_states, cos, sin, attention_mask, Wq, Wk, Wv, Wo, mode):
    mdt = _np_mmdt()
    xtb = np.asarray(hidden_states).reshape(S, D).T.astype(mdt)   # [4096, 2048]
    xblk = xtb.reshape(NT_D, 128, S)                              # [32, 128, 2048]
    cosT = np.asarray(cos).T.astype(mdt)                          # [128, 2048]
    sinT = np.asarray(sin).T.astype(mdt)
    ident = np.eye(128, dtype=mdt)
    if mode == "masked":
        msk = np.ascontiguousarray(
            np.asarray(attention_mask).reshape(S, S).T).astype(np.float32)
    else:
        msk = np.zeros((1, 1), dtype=np.float32)
    if mode == "causal":
        # 4 diagonal 0/1 tiles: tile j valid where 128*j + k <= q  (k:[128], q:[512])
        j = np.arange(4)[:, None, None]
        k = np.arange(128)[None, :, None]
        q = np.arange(512)[None, None, :]
        mflat = np.ascontiguousarray((128 * j + k <= q).astype(mdt)
                                     .transpose(1, 0, 2).reshape(128, 4 * 512))
    else:
        mflat = np.zeros((128, 4 * 512), dtype=mdt)
    ones = np.ones((128, 1), dtype=mdt)
    in_maps = []
    for c in range(NCORES):
        cc = slice(c * SPC, (c + 1) * SPC)

        def xsplit(i):
            return np.ascontiguousarray(
                xblk[_B[i]:_B[i + 1], :, cc].transpose(1, 0, 2)
                .reshape(128, N_SPLIT[i] * SPC))

        xb2 = np.empty((128, XB_W), dtype=mdt)
        xb2[:, :XB_X] = xsplit(3)
        xb2[:, CS_O:CS_O + SPC] = cosT[:, cc]
        xb2[:, SN_O:SN_O + SPC] = sinT[:, cc]
        xb2[:, MK_O:MK_O + SPC] = mflat[:, cc]
        in_maps.append({
            "xa1": xsplit(0), "xa2": xsplit(1), "xb1": xsplit(2), "xb2": xb2,
            "wq": _tile_rows(np.asarray(Wq[:, c * QC:(c + 1) * QC]).astype(mdt)),
            "wk": _tile_rows(np.asarray(Wk[:, c * DH:(c + 1) * DH]).astype(mdt)),
            "wv": _tile_rows(np.asarray(Wv[:, c * DH:(c + 1) * DH]).astype(mdt)),
            "wo": _tile_rows(np.asarray(Wo[:, c * QC:(c + 1) * QC]).astype(mdt)),
            "msk": msk, "ident": ident, "ones": ones,
        })
    return in_maps


def pick_mode(attention_mask):
    am = np.asarray(attention_mask).reshape(S, S)
    if np.array_equal(am, _causal_ref_mask()):
        return "causal"
    if not np.any(am):
        return "dense"
    return "masked"


def kernel(hidden_states, cos, sin, attention_mask, Wq, Wk, Wv, Wo, **kwargs):
    mode = pick_mode(attention_mask)
    ck = (mode, str(MMDT))
    if ck not in _CACHE:
        _CACHE[ck] = build(mode)
    nc = _CACHE[ck]
    in_maps = make_in_maps(hidden_states, cos, sin, attention_mask,
                           Wq, Wk, Wv, Wo, mode)
    res = run_bass_kernel_spmd(nc, in_maps, core_ids=list(range(NCORES)),
                               **RUN_KWARGS)
    global LAST
    LAST = res
    out = np.concatenate([res.results[c]["out"] for c in range(NCORES)], axis=1)
    return out.astype(np.float32).reshape(1, S, D)
